# revision 14
# baseline (speedup 1.0000x reference)
"""LocallyConnected2d Bass kernel for 8 TRN2 NeuronCores.

Problem: out[b,o,oh,ow] = sum_{c,kh,kw} x[b,c,oh+kh-1,ow+kw-1] * w[o,c,oh,ow,kh*3+kw]
Shapes: x (8,64,32,32) f32, weight (1,64,64,32,32,9) f32 -> out (8,64,32,32) f32.

Sharding: each core owns 4 consecutive output rows (oh); the 151 MiB weight
tensor is read exactly once, 1 byte/elem (fp8 e3m4, exact rel err 1.26e-2
vs 2e-2 tolerance), with no duplication and no collectives. The tensor
engine streams weights straight out of DMA (mixed bf16-x * fp8-w matmul);
there is no on-device dtype conversion at all.

v4 "kh-ladder" structure. Measured per-matmul wall on this part is
max(~35ns, N/5.5GHz) - small-N matmuls are issue-bound, so the kernel
maximizes N per instruction:
- One stationary x block (x row R_, column pair ow,ow+1 as [slab ;
  col-shifted slab]) serves THREE output rows at once: rung j of the MM
  computes tap pair (3j, 3j+1) of output row oh = R_ - j. M=16 packs two
  locations (ow, ow+3) of the same PE column tile; N = 2*rungs*64 <= 384.
- Taps (2,5,8) (the kw=2 column) form a second K=64 ladder on the
  un-shifted slab half, same M=16/N layout -> 102 + 102 matmuls total,
  one PE array mode switch between the phases.
- PSUM: 6 banks hold 32 sliding accumulator windows (pair (g,k) -> bank
  k, partitions 32g+8*owi+b, column 256*owi + 64*(3-oh)). start=True on
  the R_=0 matmuls marks each whole bank pending-zero (the HW has_written
  mechanism), so later matmuls accumulate with first-touch-overwrite and
  no explicit zero-init is needed. Cross-location garbage cells land in
  unused psum cells.
- Matmuls rotate PE column tiles 0/32/64 (quadrant 3 carries only the two
  leftover locations 30,31) so the three weight streams overlap (~2.3x
  measured at large N).
- All input DMAs ride the Scalar HWDGE ring in consumption order (x, then
  skewed weight tensors w1a/b/c, w2a/b/c, R-ladder-major); out rides Sync.
  The PE is DMA-paced; drains (one per output row, alternating
  Scalar/Vector) interleave into the L2 phase as rows complete.
"""

import numpy as np
import ml_dtypes

import concourse.bacc as bacc
import concourse.bass as bass
import concourse.tile as tile
from concourse import mybir
from concourse.bass_utils import run_bass_kernel_spmd

B, C, O = 8, 64, 64
OH, OW = 32, 32
NCORES = 8
R = OH // NCORES          # 4 oh rows per core
HS = R + 2                # x halo rows per core
WS = OW + 2               # padded width
F32 = mybir.dt.float32
F16 = mybir.dt.float16
BF16 = mybir.dt.bfloat16
FP8 = mybir.dt.float8e3
E3M4 = ml_dtypes.float8_e3m4

# 15 regular pairs: PE column tile t holds locations 10t..10t+9; pair
# (t, p) = adjacent locations (10t+2p, 10t+2p+1) so the stationary x slice
# is contiguous. Pair 15 = leftover locations (30, 31) on column tile 3,
# bank 5.
PAIRS16 = [(10 * t + 2 * p, 10 * t + 2 * p + 1)
           for t in range(3) for p in range(5)]
# L1 ladder: rung j = tap pair (3j, 3j+1); L2: rung j = tap 3j+2.
GRP = {0: ("a", 0), 1: ("b", 0), 2: ("c", 0),
       3: ("c", 1), 4: ("b", 1), 5: ("a", 1)}
NJ = {"a": 1, "b": 2, "c": 3}

NWARM = 8                 # N=512 warm-up matmuls (~3.5us at cold clock)

_cache: dict = {}
_last_in_maps = None


def _build() -> bass.Bass:
    nc = bacc.Bacc("TRN2", target_bir_lowering=False, debug=False,
                   num_devices=NCORES)
    # x: [0:64] = slab [c,h,w,b]; [64:128] = column-shifted duplicate.
    xa = nc.dram_tensor("xa", [128, HS, WS, B], BF16,
                        kind="ExternalInput").ap()
    # Skewed weight tensors: [p, ri, pair, owi, rung, o] (c split ri-major
    # for finer DMA pacing of the big nj=3 tensors).
    w1a = nc.dram_tensor("w1a", [128, 2, 16, 2, 1, O], FP8,
                         kind="ExternalInput").ap()
    w1b = nc.dram_tensor("w1b", [128, 2, 16, 2, 2, O], FP8,
                         kind="ExternalInput").ap()
    w1c = nc.dram_tensor("w1c", [2, 128, 16, 2, 3, O], FP8,
                         kind="ExternalInput").ap()
    w2a = nc.dram_tensor("w2a", [64, 2, 16, 2, 1, O], FP8,
                         kind="ExternalInput").ap()
    w2b = nc.dram_tensor("w2b", [64, 2, 16, 2, 2, O], FP8,
                         kind="ExternalInput").ap()
    w2c = nc.dram_tensor("w2c", [2, 64, 16, 2, 3, O], FP8,
                         kind="ExternalInput").ap()
    # out: partitions 32g+8*owi+b (regular), 96+b (leftovers); free
    # (bank k, owi-half, o) per output row.
    out = nc.dram_tensor("out", [R, 104, 6, 2, O], F16,
                         kind="ExternalOutput").ap()

    with tile.TileContext(nc) as tc:
        with (
            tc.tile_pool(name="xpool", bufs=1) as xpool,
            tc.tile_pool(name="wpool", bufs=1) as wpool,
            tc.tile_pool(name="opool", bufs=2) as opool,
            tc.tile_pool(name="pspool", bufs=1, space="PSUM") as pspool,
            tc.tile_pool(name="pwpool", bufs=1, space="PSUM") as pwpool,
        ):
            # Accumulator: [p, bank, owi-half, slot 3-oh, o] = 12KB/part.
            P = pspool.tile([128, 6, 2, 4, O], F32, name="P")
            warm = pwpool.tile([128, 512], F32, name="warm")

            x_sb = xpool.tile([128, HS, WS, B], BF16, name="x_sb")
            scr = xpool.tile([128, 512], BF16, name="scr")
            s1 = {"a": wpool.tile([128, 2, 16, 2, 1, O], FP8, name="s1a"),
                  "b": wpool.tile([128, 2, 16, 2, 2, O], FP8, name="s1b"),
                  "c": wpool.tile([128, 2, 16, 2, 3, O], FP8, name="s1c")}
            s2 = {"a": wpool.tile([128, 2, 16, 2, 1, O], FP8, name="s2a"),
                  "b": wpool.tile([128, 2, 16, 2, 2, O], FP8, name="s2b"),
                  "c": wpool.tile([128, 2, 16, 2, 3, O], FP8, name="s2c")}

            # Input stream in consumption order on the Scalar ring.
            nc.scalar.dma_start(x_sb[:], xa)
            nc.scalar.dma_start(s1["a"][:], w1a)
            nc.scalar.dma_start(s1["b"][:], w1b)
            nc.scalar.dma_start(s1["c"][:, 0], w1c[0])
            nc.scalar.dma_start(s1["c"][:, 1], w1c[1])
            nc.scalar.dma_start(s2["a"][0:64], w2a)
            nc.scalar.dma_start(s2["b"][0:64], w2b)
            nc.scalar.dma_start(s2["c"][0:64, 0], w2c[0])
            nc.scalar.dma_start(s2["c"][0:64, 1], w2c[1])

            # Zero-init every psum cell the ladder writes: start=True
            # overwrite-with-0 + set has_written (which persists across
            # NEFF executions), so every ladder matmul can accumulate
            # unconditionally. These 19 N=512 matmuls also cover the
            # x + w1a DMA wait and release the HAM clock gate.
            nc.vector.memset(scr[:], 0)
            for k in range(6):
                for t in range(3):
                    nc.tensor.matmul(P[32 * t:32 * t + 16, k, :, :, :],
                                     scr[:, 0:16], scr[:, :],
                                     start=True, stop=False,
                                     tile_position=(0, 32 * t),
                                     skip_group_check=True)
            nc.tensor.matmul(P[96:104, 5, :, :, :], scr[:, 0:8], scr[:, :],
                             start=True, stop=False, tile_position=(0, 96),
                             skip_group_check=True)

            def ladder(phase):
                for R_ in range(6):
                    grp, ri = GRP[R_]
                    nj = NJ[grp]
                    j0 = max(0, R_ - 3)
                    s0 = 3 - R_ + j0
                    sp = phase == 2 and R_ == 5
                    wt = (s1 if phase == 1 else s2)[grp]
                    # All cells were zero-initialized with has_written
                    # set, so every ladder matmul purely accumulates.
                    jsplit = [(0, nj, False)]
                    for p in range(5):
                        for t in range(3):
                            pi = t * 5 + p
                            owa = PAIRS16[pi][0]
                            if phase == 1:
                                lhsT = x_sb[:, R_, owa:owa + 2, :]
                                rhs0 = wt[:, ri, pi]
                                tp = (0, 32 * t)
                            else:
                                lhsT = x_sb[0:64, R_, owa + 2:owa + 4, :]
                                rhs0 = wt[0:64, ri, pi]
                                tp = (0, 32 * t)
                            for ja, jb, st in jsplit:
                                po = P[32 * t:32 * t + 16, p, :,
                                       s0 + ja:s0 + jb, :]
                                nc.tensor.matmul(po, lhsT,
                                                 rhs0[:, :, ja:jb],
                                                 start=st, stop=sp,
                                                 tile_position=tp,
                                                 skip_group_check=True)
                    for owi in range(2):  # leftover locs 30, 31
                        if phase == 1:
                            lhsT = x_sb[:, R_, 30 + owi, :]
                            rhs0 = wt[:, ri, 15, owi]
                            tp = (0, 96)
                        else:
                            lhsT = x_sb[0:64, R_, 32 + owi, :]
                            rhs0 = wt[0:64, ri, 15, owi]
                            tp = (0, 96)
                        for ja, jb, st in jsplit:
                            po = P[96:104, 5, owi, s0 + ja:s0 + jb, :]
                            nc.tensor.matmul(po, lhsT, rhs0[:, ja:jb],
                                             start=st, stop=sp,
                                             tile_position=tp,
                                             skip_group_check=True)
                    if phase == 2 and R_ >= 2:  # row R_-2 is complete
                        oh = R_ - 2
                        ot = opool.tile([128, 6, 2, O], F16, tag="ot")
                        src = P[:, :, :, 3 - oh, :]
                        if oh == R - 1:
                            nc.vector.tensor_copy(out=ot[0:64], in_=src[0:64])
                            nc.scalar.copy(out=ot[64:128], in_=src[64:128])
                        elif oh % 2 == 0:
                            nc.scalar.copy(out=ot[:], in_=src)
                        else:
                            nc.vector.tensor_copy(out=ot[:], in_=src)
                        nc.sync.dma_start(out[oh], ot[0:104])

            ladder(1)
            ladder(2)
    nc.compile()
    return nc


def _skew1(T, taps):
    """T: [c, oh, ow, tap, o] fp8 -> {grp: [128, 2, 16, 2, nj, O]}."""
    out = {}
    for grp, Rs in (("a", (0, 5)), ("b", (1, 4)), ("c", (2, 3))):
        nj = NJ[grp]
        arr = np.zeros((128, 2, 16, 2, nj, O), dtype=E3M4)
        for ri, R_ in enumerate(Rs):
            j0 = max(0, R_ - 3)
            for pi in range(16):
                owp = PAIRS16[pi] if pi < 15 else (30, 31)
                for owi, ow in enumerate(owp):
                    for jj in range(nj):
                        j = j0 + jj
                        oh = R_ - j
                        for half in range(2):
                            arr[64 * half:64 * half + 64, ri, pi, owi, jj] = \
                                T[:, oh, ow, taps[j][half], :]
        out[grp] = arr
    return out


def _skew2(T, taps):
    """L2: single-tap rungs on partitions 0-63."""
    out = {}
    for grp, Rs in (("a", (0, 5)), ("b", (1, 4)), ("c", (2, 3))):
        nj = NJ[grp]
        arr = np.zeros((64, 2, 16, 2, nj, O), dtype=E3M4)
        for ri, R_ in enumerate(Rs):
            j0 = max(0, R_ - 3)
            for pi in range(16):
                owp = PAIRS16[pi] if pi < 15 else (30, 31)
                for owi, ow in enumerate(owp):
                    for jj in range(nj):
                        j = j0 + jj
                        oh = R_ - j
                        arr[:, ri, pi, owi, jj] = T[:, oh, ow, taps[j][0], :]
        out[grp] = arr
    return out


def _marshal(x: np.ndarray, weight: np.ndarray) -> list[dict]:
    x = np.ascontiguousarray(x, dtype=np.float32)
    w = weight[0]  # (O, C, OH, OW, K)
    q = w.astype(E3M4)

    xs = x.astype(ml_dtypes.bfloat16)
    xp = np.zeros((B, C, OH + 2, OW + 2), dtype=ml_dtypes.bfloat16)
    xp[:, :, 1:OH + 1, 1:OW + 1] = xs

    L1TAPS = {0: (0, 1), 1: (3, 4), 2: (6, 7)}
    L2TAPS = {0: (2,), 1: (5,), 2: (8,)}  # single tap, both halves

    in_maps = []
    for r in range(NCORES):
        slab = xp[:, :, R * r:R * r + HS, :].transpose(1, 2, 3, 0)
        sw = np.zeros_like(slab)
        sw[:, :, :WS - 1, :] = slab[:, :, 1:, :]        # column shift
        xa_r = np.concatenate([slab, sw], axis=0)       # [128, HS, WS, B]

        wt = q[:, :, R * r:R * (r + 1)]                 # [o, c, 4, 32, 9]
        T = wt.transpose(1, 2, 3, 4, 0)                 # [c, oh, ow, tap, o]
        w1 = _skew1(T, L1TAPS)
        w2 = _skew2(T, L2TAPS)

        in_maps.append({
            "xa": np.ascontiguousarray(xa_r),
            "w1a": np.ascontiguousarray(w1["a"]),
            "w1b": np.ascontiguousarray(w1["b"]),
            "w1c": np.ascontiguousarray(w1["c"].transpose(1, 0, 2, 3, 4, 5)),
            "w2a": np.ascontiguousarray(w2["a"]),
            "w2b": np.ascontiguousarray(w2["b"]),
            "w2c": np.ascontiguousarray(w2["c"].transpose(1, 0, 2, 3, 4, 5)),
        })
    return in_maps


def kernel(x: np.ndarray, weight: np.ndarray) -> np.ndarray:
    global _last_in_maps
    in_maps = _marshal(x, weight)
    _last_in_maps = in_maps

    if "nc" not in _cache:
        _cache["nc"] = _build()
    res = run_bass_kernel_spmd(_cache["nc"], in_maps, list(range(NCORES)))

    # Per-core out [R, 104, 6, 2, O] f16: loc (oh, ow<30): t=ow//10,
    # p=(ow%10)//2, h=ow%2 -> partition 32t+8h+b, bank p, half h; ow=30/31
    # at partition 96+b, bank 5, half ow-30. Stitch to (B, O, OH, OW).
    full = np.empty((B, O, OH, OW), dtype=np.float32)
    for r in range(NCORES):
        o_np = np.asarray(res.results[r]["out"], dtype=np.float32)
        for oh in range(R):
            for ow in range(OW):
                if ow < 30:
                    t, p, h = ow // 10, (ow % 10) // 2, ow % 2
                    full[:, :, R * r + oh, ow] = \
                        o_np[oh, 32 * t + 8 * h:32 * t + 8 * h + B, p, h]
                else:
                    full[:, :, R * r + oh, ow] = \
                        o_np[oh, 96:96 + B, 5, ow - 30]
    return np.ascontiguousarray(full)
